# revision 25
# baseline (speedup 1.0000x reference)
"""Bass/Trainium2 kernel for nn_BehaviorSpecificPFF (MoE-style routed FFN).

Reference semantics (per token t):
    e = b_seq[t]
    out[t] = 0                                   if e == 0
    out[t] = relu(x[t] @ W1[e-1] + b1[e-1]) @ W2[e-1] + b2[e-1]   otherwise

Strategy (per core; data-parallel over batch, 4 batches = 8192 tokens/core).
Key HW facts this design is built around (all measured on TRN2):
  - tiny scattered DMA *reads* are nearly free; tiny scattered *writes* and
    CCE read-modify-write scatters are very slow,
  - gpsimd dma_gather(transpose=True) fuses gather+transpose straight into
    the matmul layout at ~1us/supertile,
  - XBAR dma_start_transpose (2-byte dtypes) runs on otherwise idle HWDGE
    queues, keeping the PE free of transposes.

Per rep:
  1. Routing scan (DVE): per-token slot in per-expert buckets (in-row prefix
     sums + matmul cross-partition prefix). Expert-0 tokens -> slot `nslot`.
  2. perm is written to DRAM contiguously and read back twice:
     (a) wrapped-16 int16 layout (the gpsimd SWDGE index format) for the
         output tail gather, and (b) the same wrapped list feeds ONE
         dma_scatter_add (on its own SWDGE queue) that builds
         sarr16[slot] = token_id in DRAM (payload id+1 into -1 init).
     sarr16 is then read back wrapped (tiny strided reads) -> gather index
     list gidxw (clamped; padding slots -> row 0).
  3. FFN per 512-slot supertile:
       - ONE dma_gather(transpose=True): x rows (bf16) by token id, written
         transposed as xt[d_chunk, tok].
       - layer 1: W1 stationary -> f32 PSUM; bias+relu fused (ACT/DVE) ->
         ht (bf16).
       - layer 2: W2 stationary -> yT [d_chunk, tok] f32 PSUM; bias via ACT
         -> yt (bf16); XBAR dma-transpose -> yo [tok, d] (SP/ACT/DVE HWDGE
         queues); ONE dense DMA write into y_bkt[slot] (bf16, double
         buffered across reps).
  4. Output tail: 8 dma_gathers read y_bkt rows by the FORWARD perm (reads,
     not scatters; expert-0 reads the zero row `nslot`) + dense writes to
     y (bf16; host converts to f32). The tail overlaps the next rep's FFN
     thanks to the y_bkt double buffer.
  - x/W1/W2/y in bf16 (tolerance 2e-2; total rel-err ~5e-3), f32 PSUM.
  - Bucket capacities specialized per call (max over cores, rounded to 128).
"""

import numpy as np

import concourse.bass as bass
import concourse.tile as tile
from concourse import bacc, mybir
from concourse.bass_utils import run_bass_kernel_spmd

N_CORES = 8
B, T, D, DFF, NB = 32, 2048, 256, 1024, 4
P = 128
NTOK = B * T // N_CORES          # 8192 tokens per core
JCOL = NTOK // P                 # 64 scan columns
F32 = mybir.dt.float32
BF16 = mybir.dt.bfloat16
I32 = mybir.dt.int32
I16 = mybir.dt.int16
AF = mybir.ActivationFunctionType
ALU = mybir.AluOpType
MD = DFF // P                    # 8 dff chunks
KD = D // P                      # 2 d_model chunks

SPLIT_SARR = 4                   # sarr16-build scatter chunks
BUILD_ELEM = 2                   # i16 payload elems per slot row
BUILD_Q = 0                      # SWDGE queue for the build scatter
TAILC = 8                        # output tail gather chunks
PIPELINED = True                 # emit scan(r+1) inside ffn(r)


def build_nc(caps, reps=1):
    """Build the per-core Bass program. caps: slot capacity per expert (mult of 128)."""
    ntiles = [c // P for c in caps]
    nslot = sum(caps)
    ntt = nslot // P                       # total 128-slot tiles
    bases = [sum(caps[:e]) for e in range(NB)]
    nsp = nslot + P                        # sarr16 / y_bkt rows (incl. zero slot)

    nc = bacc.Bacc("TRN2", target_bir_lowering=False, debug=False,
                   num_devices=N_CORES)
    x_d = nc.dram_tensor("x", [NTOK, D], BF16, kind="ExternalInput").ap()
    b_d = nc.dram_tensor("b", [NTOK], I32, kind="ExternalInput").ap()
    w1_d = nc.dram_tensor("w1s", [P, 2 * NB * DFF], BF16, kind="ExternalInput").ap()
    w2_d = nc.dram_tensor("w2s", [P, MD * NB * D], BF16, kind="ExternalInput").ap()
    b1_d = nc.dram_tensor("b1s", [P, NB * MD], F32, kind="ExternalInput").ap()
    b2_d = nc.dram_tensor("b2s", [P, NB * KD], F32, kind="ExternalInput").ap()
    y_d = nc.dram_tensor("y", [NTOK, D], BF16, kind="ExternalOutput").ap()
    perm_d = nc.dram_tensor("perm16", [NTOK], I16, kind="Internal").ap()
    sarr16 = [nc.dram_tensor(f"sarr16_{i}", [nsp, P], I16, kind="Internal").ap()
              for i in range(2)]
    y_bkt = [nc.dram_tensor(f"ybkt_{i}", [nsp, D], BF16, kind="Internal").ap()
             for i in range(2)]

    with tile.TileContext(nc) as tc:
        _body(tc, x_d, b_d, w1_d, w2_d, b1_d, b2_d, y_d, perm_d, sarr16,
              y_bkt, caps, ntiles, bases, nslot, ntt, nsp, reps)
    nc.compile()
    return nc


def _body(tc, x_d, b_d, w1_d, w2_d, b1_d, b2_d, y_d, perm_d, sarr16,
          y_bkt, caps, ntiles, bases, nslot, ntt, nsp, reps=1):
    nc = tc.nc
    ng = nc.gpsimd
    sy = nc.sync
    nv = nc.vector

    import contextlib
    ctx = contextlib.ExitStack()
    with ctx:
        const = ctx.enter_context(tc.tile_pool(name="const", bufs=1))
        scan = ctx.enter_context(tc.tile_pool(name="scan", bufs=2))
        idxp = ctx.enter_context(tc.tile_pool(name="idx", bufs=2))
        xtp = ctx.enter_context(tc.tile_pool(name="xt", bufs=4))
        htp = ctx.enter_context(tc.tile_pool(name="ht", bufs=2 * MD))
        ytp = ctx.enter_context(tc.tile_pool(name="yt", bufs=4))
        yop = ctx.enter_context(tc.tile_pool(name="yo", bufs=4))
        tlp = ctx.enter_context(tc.tile_pool(name="tl", bufs=3))
        ps_h = ctx.enter_context(tc.tile_pool(name="ps_h", bufs=3, space="PSUM"))
        ps_y = ctx.enter_context(tc.tile_pool(name="ps_y", bufs=3, space="PSUM"))

        # ---- constants / weights (once per NEFF) ---------------------------
        ltri = const.tile([P, P], F32)                 # ltri[k, m] = 1 if k < m
        ng.memset(ltri[:], 1.0)
        ng.affine_select(out=ltri[:], in_=ltri[:], compare_op=ALU.is_gt,
                         fill=0.0, base=0, pattern=[[1, P]], channel_multiplier=-1)

        w1s = const.tile([P, 2 * NB * DFF], BF16)
        sy.dma_start(w1s[:], w1_d[:])
        w2s = const.tile([P, MD * NB * D], BF16)
        sy.dma_start(w2s[:], w2_d[:])
        b1s = const.tile([P, NB * MD], F32)
        sy.dma_start(b1s[:], b1_d[:])
        b2s = const.tile([P, NB * KD], F32)
        sy.dma_start(b2s[:], b2_d[:])

        # payload for the sarr16 build: src[p, g, :] = token_id(g*128+p) + 1
        ids = const.tile([P, JCOL, BUILD_ELEM], I16)
        ng.iota(ids[:, :, :], pattern=[[P, JCOL], [0, BUILD_ELEM]], base=1,
                channel_multiplier=1, allow_small_or_imprecise_dtypes=True)
        neg1 = const.tile([P, nsp], I16)
        ng.memset(neg1[:], -1)
        zrow = const.tile([P, D], BF16)
        nv.memset(zrow[:], 0.0)
        for i in range(2):
            sy.dma_start(y_bkt[i][nslot:nsp, :], zrow[:])

        state = {}

        def scan_steps(r):
            """Generator of emission steps for rep r's scan; yields after
            each chunk so the caller can interleave with FFN supertiles."""
            st = {}
            yield lambda: st.update(permw=_scan_pre(
                tc, b_d, perm_d, sarr16[r % 2], nslot,
                scan, idxp, ps_h, ltri, neg1, bases))
            for ci in range(SPLIT_SARR):
                yield lambda ci=ci: _scan_build(
                    tc, sarr16[r % 2], ids, st["permw"], ci)
            yield lambda: st.update(gidxw=_scan_post(
                tc, sarr16[r % 2], idxp, ntt))
            yield lambda: st

        def scan_emit_all(r):
            if r >= reps:
                return None
            st = None
            for step in scan_steps(r):
                st = step()
            return st["gidxw"], st["permw"]

        cur = scan_emit_all(0)
        for r in range(reps):
            if PIPELINED and r + 1 < reps:
                steps = list(scan_steps(r + 1))
                st_box = {}

                def mk(idx):
                    def hook():
                        res = steps[idx]()
                        if idx == len(steps) - 1:
                            st_box.update(res)
                    return hook
                hooks = {1: mk(0)}
                for ci in range(SPLIT_SARR):
                    hooks[3 + 3 * ci] = mk(1 + ci)
                hooks[3 + 3 * SPLIT_SARR] = mk(1 + SPLIT_SARR)
                hooks[4 + 3 * SPLIT_SARR] = mk(2 + SPLIT_SARR)
            else:
                hooks = None
            _ffn_phase(tc, x_d, y_bkt[r % 2], caps, ntiles, bases, ntt,
                       cur, xtp, htp, ytp, yop, ps_h, ps_y,
                       w1s, w2s, b1s, b2s, hooks=hooks)
            _tail_phase(tc, y_d, y_bkt[r % 2], cur, tlp, nsp)
            if PIPELINED and r + 1 < reps:
                cur = (st_box["gidxw"], st_box["permw"])
            else:
                cur = scan_emit_all(r + 1)


def _scan_pre(tc, b_d, perm_d, sarr16, nslot,
              scan, idxp, ps_h, ltri, neg1, bases):
    """Routing scan + wrapped perm list for one rep. Returns permw."""
    nc = tc.nc
    nv = nc.vector
    ng = nc.gpsimd
    sy = nc.sync

    # re-init sarr16 to -1 (contiguous write, ~28 descriptors)
    sy.dma_start(sarr16.rearrange("(p c) o -> p (c o)", p=P), neg1[:])

    b_i = scan.tile([P, JCOL], I32)
    sy.dma_start(b_i[:], b_d.rearrange("(p j) -> p j", p=P))
    b_f = scan.tile([P, JCOL], F32)
    nv.tensor_copy(b_f[:], b_i[:])

    # masks per expert: M[p, e, j] = (b == e+1)
    M = scan.tile([P, NB * JCOL], F32)
    M3 = M[:].rearrange("p (e j) -> p e j", e=NB)
    for e in range(NB):
        nv.tensor_scalar(M3[:, e, :], b_f[:], float(e + 1), None, ALU.is_equal)

    # in-row inclusive prefix sum along j (Hillis-Steele, ping-pong)
    sA = scan.tile([P, NB * JCOL], F32)
    sB = scan.tile([P, NB * JCOL], F32)
    cur, nxt = M, sA
    s = 1
    while s < JCOL:
        c3 = cur[:].rearrange("p (e j) -> p e j", e=NB)
        n3 = nxt[:].rearrange("p (e j) -> p e j", e=NB)
        nv.tensor_copy(n3[:, :, 0:s], c3[:, :, 0:s])
        nv.tensor_add(n3[:, :, s:JCOL], c3[:, :, s:JCOL], c3[:, :, 0:JCOL - s])
        cur = nxt
        nxt = sB if cur is sA else sA
        s *= 2
    incl = cur                                        # [P, NB*JCOL]

    # per-row counts and cross-partition exclusive prefix (via matmul)
    cnt = scan.tile([P, NB], F32)
    nv.tensor_reduce(cnt[:], M3[:, :, :], mybir.AxisListType.X, ALU.add)
    exr_ps = ps_h.tile([P, NB], F32, tag="hps", name="exr_ps")
    nc.tensor.matmul(exr_ps[:], ltri[:], cnt[:], start=True, stop=True)
    exr = scan.tile([P, NB], F32)
    nv.tensor_copy(exr[:], exr_ps[:])

    # candidate slot per (token, expert); select by mask; zero-slot for e=0
    cand = scan.tile([P, NB * JCOL], F32)
    c3 = cand[:].rearrange("p (e j) -> p e j", e=NB)
    i3 = incl[:].rearrange("p (e j) -> p e j", e=NB)
    for e in range(NB):
        nv.tensor_scalar(c3[:, e, :], i3[:, e, :], exr[:, e:e + 1],
                         float(bases[e] - 1), ALU.add, ALU.add)
    prod = scan.tile([P, NB * JCOL], F32)
    nv.tensor_tensor(out=prod[:], in0=M[:], in1=cand[:], op=ALU.mult)
    perm_f = scan.tile([P, JCOL], F32)
    nv.tensor_reduce(perm_f[:],
                     prod[:].rearrange("p (e j) -> p j e", e=NB),
                     mybir.AxisListType.X, ALU.add)
    m0s = scan.tile([P, JCOL], F32)
    nv.tensor_scalar(m0s[:], b_f[:], 0.0, float(nslot), ALU.is_equal, ALU.mult)
    nv.tensor_add(perm_f[:], perm_f[:], m0s[:])
    perm16 = scan.tile([P, JCOL], I16)
    nv.tensor_copy(perm16[:], perm_f[:])

    # contiguous write of perm in token order; wrapped-16 strided read back
    sy.dma_start(perm_d.rearrange("(p j) -> p j", p=P), perm16[:])
    permw = idxp.tile([P, NTOK // 16], I16, name="permw")
    sy.dma_start(permw[0:16, :], perm_d.rearrange("(s q) -> q s", q=16))
    for r in (16, 32, 64):
        sy.dma_start(permw[r:2 * r, :], permw[0:r, :])
    return permw


def _scan_build(tc, sarr16, ids, permw, ci):
    """One sarr16-build scatter chunk (token_id+1 into -1 init)."""
    ng = tc.nc.gpsimd
    nchunk = NTOK // SPLIT_SARR
    ng.dma_scatter_add(
        out_ap=sarr16[:, 0:BUILD_ELEM],
        in_ap=ids[:, ci * (nchunk // P):(ci + 1) * (nchunk // P), :],
        idxs_ap=permw[:, ci * (nchunk // 16):(ci + 1) * (nchunk // 16)],
        num_idxs=nchunk, num_idxs_reg=nchunk, elem_size=BUILD_ELEM,
        elem_step=P, queue_num=BUILD_Q)


def _scan_post(tc, sarr16, idxp, ntt):
    """Wrapped reload of sarr16 col 0 (slot->token) + clamp. Returns gidxw."""
    nc = tc.nc
    nv = nc.vector
    sy = nc.sync
    raw = idxp.tile([P, ntt * 8, 1], I16, name="raw")
    sy.dma_start(raw[0:16, :, :],
                 sarr16.rearrange("(c q) o -> q c o", q=16)[:, :ntt * 8, 0:1])
    for r in (16, 32, 64):
        sy.dma_start(raw[r:2 * r, :, :], raw[0:r, :, :])
    gidxw = idxp.tile([P, ntt * 8], I16, name="gidxw")
    nv.tensor_scalar(gidxw[:], raw[:].rearrange("p c o -> p (c o)"),
                     0, NTOK - 1, ALU.max, ALU.min)
    return gidxw


def _ffn_phase(tc, x_d, ybkt, caps, ntiles, bases, ntt,
               idxs, xtp, htp, ytp, yop, ps_h, ps_y,
               w1s, w2s, b1s, b2s, hooks=None):
    nc = tc.nc
    nv = nc.vector
    ns = nc.scalar
    ng = nc.gpsimd
    sy = nc.sync
    gidxw, _ = idxs

    tiles = []
    for e in range(NB):
        g0 = 0
        while g0 < ntiles[e]:
            G = min(4, ntiles[e] - g0)
            tiles.append((e, bases[e] // P + g0, G))
            g0 += G

    AHEAD = 3
    store = {}

    def fetch(i):
        if i >= len(tiles):
            return
        _, t0, G = tiles[i]
        nt = G * P
        xt = xtp.tile([P, KD, nt], BF16, tag=f"xt{G}", name="xt")
        ng.dma_gather(
            out_ap=xt[:, :, :], in_ap=x_d[:],
            idxs_ap=gidxw[:, t0 * 8:t0 * 8 + G * 8],
            num_idxs=nt, num_idxs_reg=nt, elem_size=D, transpose=True)
        store[i] = xt

    for i in range(min(AHEAD, len(tiles))):
        fetch(i)

    ybkt_cov = ybkt.rearrange("(t p) d -> p t d", p=P)

    for i, (e, t0, G) in enumerate(tiles):
        nt = G * P
        xt = store.pop(i)

        # layer 1 + fused bias/relu -> ht[m][dff_chunk, tok]  (bf16 out)
        ht = [htp.tile([P, 512], BF16, tag="ht", name="ht")
              for _ in range(MD)]
        for m in range(MD):
            hps = ps_h.tile([P, 512], F32, tag="hps", name="hps")
            for k in range(KD):
                nc.tensor.matmul(
                    hps[:, :nt],
                    w1s[:, (e * KD + k) * DFF + m * P:(e * KD + k) * DFF + (m + 1) * P],
                    xt[:, k, :],
                    start=(k == 0), stop=(k == KD - 1))
            if m % 2 == 0:
                ns.activation(ht[m][:, :nt], hps[:, :nt], AF.Relu,
                              bias=b1s[:, e * MD + m:e * MD + m + 1],
                              scale=1.0)
            else:
                nv.tensor_scalar(ht[m][:, :nt], hps[:, :nt],
                                 b1s[:, e * MD + m:e * MD + m + 1],
                                 0.0, ALU.add, ALU.max)

        # layer 2, W2 stationary -> yT[d_chunk, tok]; bias on ACT -> bf16
        yt = [ytp.tile([P, 512], BF16, tag="yt", name="yt")
              for _ in range(KD)]
        for c in range(KD):
            yps = ps_y.tile([P, 512], F32, tag="yps", name="yps")
            for f in range(MD):
                nc.tensor.matmul(
                    yps[:, :nt],
                    w2s[:, (e * MD + f) * D + c * P:(e * MD + f) * D + (c + 1) * P],
                    ht[f][:, :nt],
                    start=(f == 0), stop=(f == MD - 1))
            ns.activation(yt[c][:, :nt], yps[:, :nt], AF.Identity,
                          bias=b2s[:, e * KD + c:e * KD + c + 1], scale=1.0)

        # XBAR transpose yT -> yo [tok, d] (bf16) on HWDGE queues
        yo = yop.tile([P, 4, D], BF16, name="yo")
        engs = [sy, sy, sy, ns, sy, sy, sy, ns]
        for gi in range(G):
            for c in range(KD):
                engs[(gi * KD + c) % len(engs)].dma_start_transpose(
                    yo[:, gi, c * P:(c + 1) * P],
                    yt[c][:, gi * P:(gi + 1) * P])

        # prefetch, then ONE dense write into y_bkt slots
        fetch(i + AHEAD)
        sy.dma_start(ybkt_cov[:, t0:t0 + G, :], yo[:, :G, :])

        if hooks and i in hooks:
            hooks.pop(i)()

    if hooks:
        for i in sorted(hooks):
            hooks.pop(i)()


def _tail_phase(tc, y_d, ybkt, idxs, tlp, nsp):
    """Gather y_bkt rows by forward perm -> y (token order), in chunks."""
    nc = tc.nc
    ng = nc.gpsimd
    sy = nc.sync
    _, permw = idxs
    ctok = NTOK // TAILC
    for c in range(TAILC):
        yg = tlp.tile([P, ctok // P, D], BF16, tag="yg", name="yg")
        ng.dma_gather(
            out_ap=yg[:, :, :], in_ap=ybkt[:],
            idxs_ap=permw[:, c * (ctok // 16):(c + 1) * (ctok // 16)],
            num_idxs=ctok, num_idxs_reg=ctok, elem_size=D, transpose=False)
        sy.dma_start(
            y_d.rearrange("(c j p) d -> c p j d", c=TAILC, p=P)[c],
            yg[:, :, :])


def prep_inputs(x, W1, b1, W2, b2, b_seq):
    """Shard + pre-layout host-side. Returns (in_maps, caps)."""
    import ml_dtypes
    bf16 = ml_dtypes.bfloat16
    x = np.asarray(x, dtype=np.float32)
    W1 = np.asarray(W1, dtype=np.float32)
    b1 = np.asarray(b1, dtype=np.float32)
    W2 = np.asarray(W2, dtype=np.float32)
    b2 = np.asarray(b2, dtype=np.float32)
    b_seq = np.ascontiguousarray(np.asarray(b_seq, dtype=np.int32))

    w1s = np.ascontiguousarray(
        W1.reshape(NB, 2, P, DFF).transpose(2, 0, 1, 3).reshape(P, 2 * NB * DFF)
    ).astype(bf16)
    w2s = np.ascontiguousarray(
        W2.reshape(NB, MD, P, D).transpose(2, 0, 1, 3).reshape(P, -1)
    ).astype(bf16)
    b1s = np.ascontiguousarray(
        b1.reshape(NB, MD, P).transpose(2, 0, 1).reshape(P, -1))
    b2s = np.ascontiguousarray(
        b2.reshape(NB, KD, P).transpose(2, 0, 1).reshape(P, -1))

    bpc = B // N_CORES
    in_maps = []
    counts = np.zeros((N_CORES, NB), dtype=np.int64)
    for c in range(N_CORES):
        xc = x[c * bpc:(c + 1) * bpc].reshape(NTOK, D).astype(bf16)
        bc = b_seq[c * bpc:(c + 1) * bpc].reshape(NTOK)
        for e in range(NB):
            counts[c, e] = int((bc == e + 1).sum())
        in_maps.append({"x": np.ascontiguousarray(xc),
                        "b": np.ascontiguousarray(bc),
                        "w1s": w1s, "w2s": w2s, "b1s": b1s, "b2s": b2s})
    caps = [max(P, int(np.ceil(counts[:, e].max() / P)) * P) for e in range(NB)]
    return in_maps, caps


def assemble(results):
    bpc = B // N_CORES
    out = np.empty((B, T, D), dtype=np.float32)
    for c in range(N_CORES):
        out[c * bpc:(c + 1) * bpc] = (
            results[c]["y"].astype(np.float32).reshape(bpc, T, D))
    return out


def kernel(x, W1, b1, W2, b2, b_seq):
    in_maps, caps = prep_inputs(x, W1, b1, W2, b2, b_seq)
    nc = build_nc(caps)
    res = run_bass_kernel_spmd(nc, in_maps, core_ids=list(range(N_CORES)))
    return assemble(res.results)


# revision 28
# speedup vs baseline: 2.5726x; 2.5726x over previous
"""Bass/Trainium2 kernel for nn_BehaviorSpecificPFF (MoE-style routed FFN).

Reference semantics (per token t):
    e = b_seq[t]
    out[t] = 0                                   if e == 0
    out[t] = relu(x[t] @ W1[e-1] + b1[e-1]) @ W2[e-1] + b2[e-1]   otherwise

Strategy:
  - Data parallel over batch: 32 batches -> 4 per core on 8 cores.
  - Per core (8192 tokens), entirely on device:
      1. Routing scan: from b_seq compute, for every token, a unique slot in a
         per-expert bucket (matmul-based cross-partition prefix sum + shifted-add
         in-row prefix sum). Scatter token ids into two DRAM index arrays
         (gather-index, init 0; scatter-index, init BIG so padding slots are
         dropped by the bounds check).
      2. For each expert bucket, in supertiles of up to 512 slots: indirect-DMA
         gather x rows, PE-transpose to [d, tok], two matmul layers (fp32 data,
         fp32r matmul mode) with bias+relu fused on the ACT engine, PE-transpose
         back to [tok, d], indirect-DMA scatter rows to the output (padding slots
         dropped via bounds check; expert-0 rows stay zero from the zero-init
         output buffer).
  - Bucket capacities are specialized per call (max over cores, rounded to 128);
    the kernel is otherwise input-agnostic.
"""

import numpy as np

import concourse.bass as bass
import concourse.tile as tile
from concourse import bacc, mybir
from concourse.bass import IndirectOffsetOnAxis
from concourse.bass_utils import run_bass_kernel_spmd
from concourse.masks import make_identity

N_CORES = 8
B, T, D, DFF, NB = 32, 2048, 256, 1024, 4
P = 128
NTOK = B * T // N_CORES          # 8192 tokens per core
JCOL = NTOK // P                 # 64 scan columns
BIG = 100000
F32 = mybir.dt.float32
F32R = mybir.dt.float32r
I32 = mybir.dt.int32
AF = mybir.ActivationFunctionType
ALU = mybir.AluOpType


def build_nc(caps, mm_dtype=F32R, debug=False, reps=1, parts=("scan", "gather", "mm", "scatter")):
    """Build the per-core Bass program. caps: slot capacity per expert (mult of 128)."""
    ntiles = [c // P for c in caps]
    nslot = sum(caps)
    ntt = nslot // P                       # total 128-slot tiles
    bases = [sum(caps[:e]) for e in range(NB)]

    nc = bacc.Bacc("TRN2", target_bir_lowering=False, debug=False,
                   num_devices=N_CORES)
    x_d = nc.dram_tensor("x", [NTOK, D], F32, kind="ExternalInput").ap()
    b_d = nc.dram_tensor("b", [NTOK], I32, kind="ExternalInput").ap()
    w1_d = nc.dram_tensor("w1s", [P, 2 * NB * DFF], F32R, kind="ExternalInput").ap()
    w2_d = nc.dram_tensor("w2s", [P, (DFF // P) * NB * D], F32R, kind="ExternalInput").ap()
    b1_d = nc.dram_tensor("b1s", [P, NB * (DFF // P)], F32, kind="ExternalInput").ap()
    b2_d = nc.dram_tensor("b2s", [P, NB * (D // P)], F32, kind="ExternalInput").ap()
    y_d = nc.dram_tensor("y", [NTOK, D], F32, kind="ExternalOutput").ap()
    sarr = nc.dram_tensor("sarr", [nslot, 1], I32, kind="Internal").ap()

    with tile.TileContext(nc) as tc:
        _body(tc, x_d, b_d, w1_d, w2_d, b1_d, b2_d, y_d, sarr,
              caps, ntiles, bases, nslot, ntt, mm_dtype, None, reps, parts)
    nc.compile()
    return nc


def _body(tc, x_d, b_d, w1_d, w2_d, b1_d, b2_d, y_d, sarr,
          caps, ntiles, bases, nslot, ntt, mm_dtype, dbg=None, reps=1,
          parts=("scan", "gather", "mm", "scatter")):
    nc = tc.nc
    nv = nc.vector
    ns = nc.scalar
    ng = nc.gpsimd
    sy = nc.sync

    import contextlib
    ctx = contextlib.ExitStack()
    with ctx:
        const = ctx.enter_context(tc.tile_pool(name="const", bufs=1))
        scan = ctx.enter_context(tc.tile_pool(name="scan", bufs=1))
        idxp = ctx.enter_context(tc.tile_pool(name="idx", bufs=4))
        xgp = ctx.enter_context(tc.tile_pool(name="xg", bufs=4))
        xtp = ctx.enter_context(tc.tile_pool(name="xt", bufs=6))
        htp = ctx.enter_context(tc.tile_pool(name="ht", bufs=12))
        ytp = ctx.enter_context(tc.tile_pool(name="yt", bufs=6))
        yop = ctx.enter_context(tc.tile_pool(name="yo", bufs=4))
        ps_int = ctx.enter_context(tc.tile_pool(name="ps_int", bufs=3, space="PSUM"))
        ps_h = ctx.enter_context(tc.tile_pool(name="ps_h", bufs=3, space="PSUM"))
        ps_y = ctx.enter_context(tc.tile_pool(name="ps_y", bufs=2, space="PSUM"))
        ps_outt = ps_int

        # ---- constants / weights -------------------------------------------
        ident = const.tile([P, P], F32)
        make_identity(nc, ident[:])
        ltri = const.tile([P, P], F32)                 # ltri[k, m] = 1 if k < m
        ng.memset(ltri[:], 1.0)
        ng.affine_select(out=ltri[:], in_=ltri[:], compare_op=ALU.is_gt,
                         fill=0.0, base=0, pattern=[[1, P]], channel_multiplier=-1)

        w1s = const.tile([P, 2 * NB * DFF], F32R)
        sy.dma_start(w1s[:], w1_d[:])
        w2s = const.tile([P, (DFF // P) * NB * D], F32R)
        sy.dma_start(w2s[:], w2_d[:])
        b1s = const.tile([P, NB * (DFF // P)], F32)
        sy.dma_start(b1s[:], b1_d[:])
        b2s = const.tile([P, NB * (D // P)], F32)
        sy.dma_start(b2s[:], b2_d[:])

        # ---- init index arrays in DRAM -------------------------------------
        bt = const.tile([P, ntt], I32)
        ng.memset(bt[:], BIG)
        sarr_cov = sarr.rearrange("(p t) o -> p (t o)", p=P)
        sy.dma_start(sarr_cov[:, :], bt[:])

        # ---- phase 1: routing scan -----------------------------------------
        for _rep in range(reps):
            _phases(tc, x_d, b_d, y_d, sarr, caps, ntiles, bases, nslot, ntt,
                    scan, idxp, xgp, xtp, htp, ytp, yop,
                    ps_int, ps_h, ps_y, ps_outt,
                    ident, ltri, w1s, w2s, b1s, b2s, None,
                    parts)


def _phases(tc, x_d, b_d, y_d, sarr, caps, ntiles, bases, nslot, ntt,
            scan, idxp, xgp, xtp, htp, ytp, yop,
            ps_int, ps_h, ps_y, ps_outt,
            ident, ltri, w1s, w2s, b1s, b2s, dbg=None,
            parts=("scan", "gather", "mm", "scatter")):
        if "scan" in parts:
            _scan_phase(tc, b_d, sarr, bases, nslot, scan, ps_h, ltri, dbg)
        _ffn_phase(tc, x_d, y_d, sarr, caps, ntiles, bases, nslot, ntt,
                   idxp, xgp, xtp, htp, ytp, yop, ps_int, ps_h, ps_y, ps_outt,
                   ident, w1s, w2s, b1s, b2s, parts)


def _scan_phase(tc, b_d, sarr, bases, nslot, scan, ps_h, ltri, dbg=None):
        nc = tc.nc
        nv = nc.vector
        ng = nc.gpsimd
        sy = nc.sync

        b_i = scan.tile([P, JCOL], I32)
        sy.dma_start(b_i[:], b_d.rearrange("(p j) -> p j", p=P))
        b_f = scan.tile([P, JCOL], F32)
        nv.tensor_copy(b_f[:], b_i[:])

        # masks per expert: M[p, e, j] = (b == e+1)
        M = scan.tile([P, NB * JCOL], F32)
        M3 = M[:].rearrange("p (e j) -> p e j", e=NB)
        for e in range(NB):
            nv.tensor_scalar(M3[:, e, :], b_f[:], float(e + 1), None, ALU.is_equal)

        # in-row inclusive prefix sum along j (Hillis-Steele, ping-pong)
        sA = scan.tile([P, NB * JCOL], F32)
        sB = scan.tile([P, NB * JCOL], F32)
        cur, nxt = M, sA
        s = 1
        while s < JCOL:
            c3 = cur[:].rearrange("p (e j) -> p e j", e=NB)
            n3 = nxt[:].rearrange("p (e j) -> p e j", e=NB)
            nv.tensor_copy(n3[:, :, 0:s], c3[:, :, 0:s])
            nv.tensor_add(n3[:, :, s:JCOL], c3[:, :, s:JCOL], c3[:, :, 0:JCOL - s])
            cur = nxt
            nxt = sB if cur is sA else sA
            s *= 2
        incl = cur                                        # [P, NB*JCOL]

        # per-row counts and cross-partition exclusive prefix (via matmul)
        cnt = scan.tile([P, NB], F32)
        nv.tensor_reduce(cnt[:], M3[:, :, :], mybir.AxisListType.X, ALU.add)
        exr_ps = ps_h.tile([P, NB], F32, tag="hps", name="exr_ps")
        nc.tensor.matmul(exr_ps[:], ltri[:], cnt[:], start=True, stop=True)
        exr = scan.tile([P, NB], F32)
        nv.tensor_copy(exr[:], exr_ps[:])

        # candidate slot per (token, expert); select by mask; BIG for expert 0
        cand = scan.tile([P, NB * JCOL], F32)
        c3 = cand[:].rearrange("p (e j) -> p e j", e=NB)
        i3 = incl[:].rearrange("p (e j) -> p e j", e=NB)
        for e in range(NB):
            nv.tensor_scalar(c3[:, e, :], i3[:, e, :], exr[:, e:e + 1],
                             float(bases[e] - 1), ALU.add, ALU.add)
        prod = scan.tile([P, NB * JCOL], F32)
        nv.tensor_tensor(out=prod[:], in0=M[:], in1=cand[:], op=ALU.mult)
        perm_f = scan.tile([P, JCOL], F32)
        nv.tensor_reduce(perm_f[:],
                         prod[:].rearrange("p (e j) -> p j e", e=NB),
                         mybir.AxisListType.X, ALU.add)
        m0s = scan.tile([P, JCOL], F32)
        nv.tensor_scalar(m0s[:], b_f[:], 0.0, float(BIG), ALU.is_equal, ALU.mult)
        nv.tensor_add(perm_f[:], perm_f[:], m0s[:])
        perm_i = scan.tile([P, JCOL], I32)
        nv.tensor_copy(perm_i[:], perm_f[:])

        tid = scan.tile([P, JCOL], I32)
        ng.iota(tid[:], pattern=[[1, JCOL]], base=0, channel_multiplier=JCOL)

        for j in range(JCOL):
            ng.indirect_dma_start(
                out=sarr[:], out_offset=IndirectOffsetOnAxis(ap=perm_i[:, j:j + 1], axis=0),
                in_=tid[:, j:j + 1], in_offset=None,
                bounds_check=nslot - 1, oob_is_err=False)


def _ffn_phase(tc, x_d, y_d, sarr, caps, ntiles, bases, nslot, ntt,
               idxp, xgp, xtp, htp, ytp, yop, ps_int, ps_h, ps_y, ps_outt,
               ident, w1s, w2s, b1s, b2s, parts):
    nc = tc.nc
    nv = nc.vector
    ns = nc.scalar
    ng = nc.gpsimd
    sy = nc.sync
    sarr_t = sarr.rearrange("(t p) o -> p (t o)", p=P)

    tiles = []
    for e in range(NB):
        g0 = 0
        while g0 < ntiles[e]:
            G = min(4, ntiles[e] - g0)
            tiles.append((e, bases[e] // P + g0, G))
            g0 += G

    FETCH_AHEAD = 2
    store = {}

    def fetch(i):
        if i >= len(tiles):
            return
        _, t0, G = tiles[i]
        idst = idxp.tile([P, 4], I32, tag="idst", name="idst")
        sy.dma_start(idst[:, :G], sarr_t[:, t0:t0 + G])
        isrc = idxp.tile([P, 4], I32, tag="isrc", name="isrc")
        nv.tensor_scalar(isrc[:, :G], idst[:, :G], NTOK - 1, None, ALU.min)
        xg = xgp.tile([P, 4 * D], F32, name="xg")
        if "gather" in parts:
            for gi in range(G):
                ng.indirect_dma_start(
                    out=xg[:, gi * D:(gi + 1) * D], out_offset=None,
                    in_=x_d[:],
                    in_offset=IndirectOffsetOnAxis(ap=isrc[:, gi:gi + 1], axis=0))
        elif "mm" in parts:
            nv.memset(xg[:], 0.0)
        store[i] = (idst, isrc, xg)

    for i in range(min(FETCH_AHEAD, len(tiles))):
        fetch(i)

    for i, (e, t0, G) in enumerate(tiles):
        ntoks = G * P
        idst, isrc, xg = store.pop(i)

        yo = yop.tile([P, 4 * D], F32)
        if "scatter" in parts and "mm" not in parts:
            nv.memset(yo[:], 0.0)
        if "mm" in parts:
            # transpose gathered [tok, d] -> xt[k][d_chunk, tok]
            xt = [xtp.tile([P, 512], F32R, tag=f"xt{k}", name=f"xt{k}")
                  for k in range(2)]
            for k in range(2):
                pst = ps_int.tile([P, 512], F32, tag="pst", name="pst")
                for gi in range(G):
                    nc.tensor.transpose(
                        out=pst[:, gi * P:(gi + 1) * P],
                        in_=xg[:, gi * D + k * P: gi * D + (k + 1) * P],
                        identity=ident[:])
                nv.tensor_copy(xt[k][:, :ntoks], pst[:, :ntoks])

            # layer 1 + fused bias/relu -> ht[m][dff_chunk, tok]
            ht = [htp.tile([P, 512], F32R, tag="ht", name="ht")
                  for _ in range(DFF // P)]
            for m in range(DFF // P):
                hps = ps_h.tile([P, 512], F32)
                for k in range(2):
                    nc.tensor.matmul(
                        hps[:, :ntoks],
                        w1s[:, (e * 2 + k) * DFF + m * P:(e * 2 + k) * DFF + (m + 1) * P],
                        xt[k][:, :ntoks],
                        start=(k == 0), stop=(k == 1))
                if m % 2 == 0:
                    ns.activation(ht[m][:, :ntoks], hps[:, :ntoks], AF.Relu,
                                  bias=b1s[:, e * (DFF // P) + m:e * (DFF // P) + m + 1],
                                  scale=1.0)
                else:
                    nv.tensor_scalar(ht[m][:, :ntoks], hps[:, :ntoks],
                                     b1s[:, e * (DFF // P) + m:e * (DFF // P) + m + 1],
                                     0.0, ALU.add, ALU.max)

            # layer 2 + bias -> yt[c][dmodel_chunk, tok]
            yt = [ytp.tile([P, 512], F32, tag="yt", name="yt")
                  for _ in range(D // P)]
            for c in range(D // P):
                yps = ps_y.tile([P, 512], F32)
                for k in range(DFF // P):
                    nc.tensor.matmul(
                        yps[:, :ntoks],
                        w2s[:, (e * (DFF // P) + k) * D + c * P:(e * (DFF // P) + k) * D + (c + 1) * P],
                        ht[k][:, :ntoks],
                        start=(k == 0), stop=(k == DFF // P - 1))
                nv.tensor_scalar(yt[c][:, :ntoks], yps[:, :ntoks],
                                 b2s[:, e * (D // P) + c:e * (D // P) + c + 1],
                                 None, ALU.add)

            # transpose back [dmodel, tok] -> yo[tok, dmodel], 2 blocks/pack
            for pk in range((G + 1) // 2):
                gis = [gi for gi in (2 * pk, 2 * pk + 1) if gi < G]
                pso = ps_outt.tile([P, 512], F32, tag="pst", name="pso")
                for bi, gi in enumerate(gis):
                    for c in range(D // P):
                        nc.tensor.transpose(
                            out=pso[:, bi * D + c * P: bi * D + (c + 1) * P],
                            in_=yt[c][:, gi * P:(gi + 1) * P],
                            identity=ident[:])
                nv.tensor_copy(yo[:, 2 * pk * D: (2 * pk + len(gis)) * D],
                               pso[:, :len(gis) * D])

        # prefetch the supertile two ahead BEFORE this tile's scatters, so
        # the in-order POOL queue runs its gathers without waiting on our mm
        fetch(i + FETCH_AHEAD)

        if "scatter" in parts:
            sidx = idst if "scan" in parts else isrc
            for gi in range(G):
                ng.indirect_dma_start(
                    out=y_d[:],
                    out_offset=IndirectOffsetOnAxis(ap=sidx[:, gi:gi + 1], axis=0),
                    in_=yo[:, gi * D:(gi + 1) * D], in_offset=None,
                    bounds_check=NTOK - 1, oob_is_err=False)


def prep_inputs(x, W1, b1, W2, b2, b_seq):
    """Shard + pre-layout host-side. Returns (in_maps, caps)."""
    x = np.ascontiguousarray(np.asarray(x, dtype=np.float32))
    W1 = np.asarray(W1, dtype=np.float32)
    b1 = np.asarray(b1, dtype=np.float32)
    W2 = np.asarray(W2, dtype=np.float32)
    b2 = np.asarray(b2, dtype=np.float32)
    b_seq = np.ascontiguousarray(np.asarray(b_seq, dtype=np.int32))

    w1s = np.ascontiguousarray(
        W1.reshape(NB, 2, P, DFF).transpose(2, 0, 1, 3).reshape(P, 2 * NB * DFF))
    w2s = np.ascontiguousarray(
        W2.reshape(NB, DFF // P, P, D).transpose(2, 0, 1, 3).reshape(P, -1))
    b1s = np.ascontiguousarray(
        b1.reshape(NB, DFF // P, P).transpose(2, 0, 1).reshape(P, -1))
    b2s = np.ascontiguousarray(
        b2.reshape(NB, D // P, P).transpose(2, 0, 1).reshape(P, -1))

    bpc = B // N_CORES
    in_maps = []
    counts = np.zeros((N_CORES, NB), dtype=np.int64)
    for c in range(N_CORES):
        xc = x[c * bpc:(c + 1) * bpc].reshape(NTOK, D)
        bc = b_seq[c * bpc:(c + 1) * bpc].reshape(NTOK)
        for e in range(NB):
            counts[c, e] = int((bc == e + 1).sum())
        in_maps.append({"x": np.ascontiguousarray(xc),
                        "b": np.ascontiguousarray(bc),
                        "w1s": w1s, "w2s": w2s, "b1s": b1s, "b2s": b2s})
    caps = [max(P, int(np.ceil(counts[:, e].max() / P)) * P) for e in range(NB)]
    return in_maps, caps


def assemble(results):
    bpc = B // N_CORES
    out = np.empty((B, T, D), dtype=np.float32)
    for c in range(N_CORES):
        out[c * bpc:(c + 1) * bpc] = results[c]["y"].reshape(bpc, T, D)
    return out


def kernel(x, W1, b1, W2, b2, b_seq):
    in_maps, caps = prep_inputs(x, W1, b1, W2, b2, b_seq)
    nc = build_nc(caps)
    res = run_bass_kernel_spmd(nc, in_maps, core_ids=list(range(N_CORES)))
    return assemble(res.results)
